# revision 25
# baseline (speedup 1.0000x reference)
"""LoRA self-attention Trainium2 kernel, 8-way head/tensor parallel.

Single software-pipelined stream (no separate projection phase): the ACT
exp stream (16 exps of [128,1024] per block, ~1.0us each) is the pacing
resource in steady state; everything else fills PE/DVE/DMA slack around it.

Sharding: core c owns heads 2c, 2c+1 (channels 128c..128c+128) for the
QKV projections and attention; the output projection is token-sharded
(core c computes all 1024 output channels for tokens 512c..512c+512)
after an AllToAll exchange of the attention output.

Design:
- LoRA folded into the dense weights on host (W_eff = W + 2*B@A, exact).
- Prologue projects K,Q,V(j0)+K,V(j1) in parallel PSUM banks as soon as
  the first x chunks land; the attention block loop starts right after.
- Remaining projections (KV j2..j7, Q j1..j7) are JIT-interleaved into the
  block loop's PE slots through a single rotating PSUM bank (K->V->Q).
- Softmax denominator free via the ones column in the augmented-V layout
  (M=65 AV matmuls: attn@V in psum partitions 0..63, denominator in 64).
- Block normalization deferred into the next block: reciprocals on DVE,
  replicated across partitions via gpsimd partition_broadcast (frees the
  PSUM bank the old replicate-matmul needed), multiply on DVE, ship.
- bf16 everywhere (fp8 was measured to blow the 2e-2 error budget).
- Output projection computed transposed (tokens on partitions), bias added
  from a host-broadcast [128,1024] tile on DVE during PSUM eviction.
"""
import sys

for p in ("/opt/trn_rl_repo",):
    if p not in sys.path:
        sys.path.append(p)

import numpy as np

import concourse.bass as bass  # noqa: F401
import concourse.tile as tile
from concourse import bacc, mybir
from concourse import bass_utils

N_CORES = 8
EMBED = 1024
HEADS = 16
HD = 64            # head dim
NB = 2             # batch
S = 2048           # seq len
T = NB * S         # 4096 tokens
CH = EMBED // N_CORES  # 128 channels (2 heads) per core
FP = mybir.dt.float32
BF = mybir.dt.bfloat16
AF = mybir.ActivationFunctionType
BF_NP = mybir.dt.np(mybir.dt.bfloat16)

_CACHE: dict = {}

NE = EMBED // 128  # 8 contraction tiles
NJ = T // 512      # 8 token tiles


def _build(local_only=False, dbg=False):
    nc = bacc.Bacc("TRN2", target_bir_lowering=False, debug=False,
                   enable_asserts=False, num_devices=N_CORES)
    if dbg:
        DQT = nc.dram_tensor("DQT", [128, T], BF, kind="ExternalOutput").ap()
        DKT = nc.dram_tensor("DKT", [128, T], BF, kind="ExternalOutput").ap()
        DVS = nc.dram_tensor("DVS", [128, 32 * 130], BF, kind="ExternalOutput").ap()
        DBIN = nc.dram_tensor("DBIN", [N_CORES, 128, 512], BF, kind="ExternalOutput").ap()
        DATT = nc.dram_tensor("DATT", [128, NE * 512], BF, kind="ExternalOutput").ap()
    # ---- DRAM I/O (per-core) ----
    # x pre-arranged on host: [128, (j e t)] = [128, 8*8*512]
    xP = nc.dram_tensor("xP", [128, NJ * NE * 512], BF, kind="ExternalInput").ap()
    # packed weights [wk | wq | wv] each [128, (e c)] = [128, 8*128]
    wP = nc.dram_tensor("wP", [128, 3 * NE * CH], BF, kind="ExternalInput").ap()
    bias3 = nc.dram_tensor("bias3", [CH, 3], FP, kind="ExternalInput").ap()
    ident = nc.dram_tensor("ident", [128, 128], BF, kind="ExternalInput").ap()
    # Wo pre-packed on host: [128, (ci o)] with [p, ci*1024+o] = Wo_eff.T[ci*128+p, o]
    woP = nc.dram_tensor("woP", [128, NE * EMBED], BF, kind="ExternalInput").ap()
    # output bias broadcast to all 128 partitions on host
    boB = nc.dram_tensor("boB", [128, EMBED], BF, kind="ExternalInput").ap()
    Y = nc.dram_tensor("Y", [512, EMBED], BF, kind="ExternalOutput").ap()

    with tile.TileContext(nc) as tc, \
         nc.allow_low_precision(reason="bf16 rounding is intentional"):
        with tc.tile_pool(name="const", bufs=1) as cpool, \
             tc.tile_pool(name="big", bufs=1) as bigpool, \
             tc.tile_pool(name="dram", bufs=1, space="DRAM") as dram:

            # ---- resident tiles ----
            w_all = cpool.tile([128, 3 * NE * CH], BF, tag="wall")
            wk_sb = w_all[:, 0:NE * CH]
            wq_sb = w_all[:, NE * CH:2 * NE * CH]
            wv_sb = w_all[:, 2 * NE * CH:3 * NE * CH]
            x_sb = bigpool.tile([128, NJ * NE * 512], BF, tag="x")
            bias_sb = cpool.tile([CH, 3], FP, tag="bias3")
            id_sb = cpool.tile([128, 128], BF, tag="ident")
            bo_sb = cpool.tile([128, EMBED], BF, tag="boB")
            wo_sb = cpool.tile([128, NE * EMBED], BF, tag="wo")

            QT_sb = bigpool.tile([CH, T], BF, tag="QT")
            KT_sb = bigpool.tile([CH, T], BF, tag="KT")
            VT_sb = bigpool.tile([CH, T], BF, tag="VT")
            # V in [token, ch] layout, 32 strips of [128, 130]:
            # cols [s*130+h*65 : +64] = V head h, col [s*130+h*65+64] = ones
            V_sb = bigpool.tile([128, 32 * 130], BF, tag="Vaug")

            def xs(j, e0, e1):
                return slice((j * NE + e0) * 512, (j * NE + e1) * 512)

            def xt(j, e):
                return x_sb[:, (j * NE + e) * 512:(j * NE + e + 1) * 512]

            # ---- head DMAs: feed the first matmuls ASAP ----
            nc.sync.dma_start(w_all[:, 0:NE * CH], wP[:, 0:NE * CH])          # wk
            nc.sync.dma_start(x_sb[:, xs(0, 0, 4)], xP[:, xs(0, 0, 4)])
            nc.sync.dma_start(bias_sb[:], bias3)
            nc.sync.dma_start(w_all[:, NE * CH:2 * NE * CH],
                              wP[:, NE * CH:2 * NE * CH])                     # wq
            nc.sync.dma_start(x_sb[:, xs(0, 4, 8)], xP[:, xs(0, 4, 8)])
            nc.sync.dma_start(w_all[:, 2 * NE * CH:3 * NE * CH],
                              wP[:, 2 * NE * CH:3 * NE * CH])                 # wv
            nc.sync.dma_start(id_sb[:], ident)
            nc.sync.dma_start(x_sb[:, xs(1, 0, 8)], xP[:, xs(1, 0, 8)])

            # ones columns of the augmented-V layout (all strips, once)
            v_ones = V_sb.rearrange("p (s c) -> p s c", c=65)[:, :, 64]
            nc.vector.memset(v_ones, 1.0)

            bounce_in = dram.tile([N_CORES, 128, 512], BF)
            bounce_out = dram.tile([N_CORES, 128, 512], BF)

            with tc.tile_pool(name="psC", bufs=2, space="PSUM") as psC, \
                 tc.tile_pool(name="psO", bufs=1, space="PSUM") as psO, \
                 tc.tile_pool(name="psP", bufs=1, space="PSUM") as psP, \
                 tc.tile_pool(name="psT", bufs=1, space="PSUM") as psT, \
                 tc.tile_pool(name="pt", bufs=12) as ptpool, \
                 tc.tile_pool(name="rs", bufs=2) as rpool:

                # shared transpose scratch: 8 slots of [128,128]bf16 in one
                # PSUM bank; strip t uses slot t%8 so adjacent j-tiles don't
                # collide
                tr_ps = psT.tile([128, 1024], BF, tag="tr")

                # ---------- projection machinery (shared PSUM bank) ----------
                W_OF = {"k": wk_sb, "q": wq_sb, "v": wv_sb}
                B_OF = {"q": 0, "k": 1, "v": 2}
                DST = {"k": KT_sb, "q": QT_sb, "v": VT_sb}
                pp = {"t": None}

                def proj_mm(which, j, e):
                    if e == 0:
                        pp["t"] = psP.tile([CH, 512], FP, tag="p", name="pp")
                    w = W_OF[which]
                    nc.tensor.matmul(pp["t"][:], w[:, e * CH:(e + 1) * CH],
                                     xt(j, e), start=(e == 0), stop=(e == NE - 1))
                    if e == NE - 1:
                        t0 = j * 512
                        nc.vector.tensor_scalar_add(
                            DST[which][:, t0:t0 + 512], pp["t"][:],
                            bias_sb[:, B_OF[which]:B_OF[which] + 1])

                def vtr(j, si):
                    # one V strip -> augmented [token, ch] layout: PE-mode
                    # transpose into the shared psT slot, two DVE copies out
                    t = j * 4 + si
                    sl = (t % 8) * 128
                    trp = tr_ps[:, sl:sl + 128]
                    nc.tensor.transpose(trp, VT_sb[:, t * 128:(t + 1) * 128],
                                        id_sb[:])
                    base = t * 130
                    nc.vector.tensor_copy(V_sb[:, base:base + 64], trp[:, 0:64])
                    nc.vector.tensor_copy(V_sb[:, base + 65:base + 129],
                                          trp[:, 64:128])

                def proj_units(*specs):
                    # specs: (which, j) -> per-e emitters; None spacer after
                    # each group lets the PSUM-bank WAR (evict on DVE) clear
                    # before the next group's start=True matmul; V groups are
                    # followed by the 4 strip transposes
                    out = []
                    for which, j in specs:
                        for e in range(NE):
                            out.append(lambda which=which, j=j, e=e:
                                       proj_mm(which, j, e))
                        out.append(None)
                        if which == "v":
                            for si in range(4):
                                out.append(lambda j=j, si=si: vtr(j, si))
                    return out

                # ---------- prologue: j0 (K,Q,V) + j1 (K,V) ----------
                # parallel PSUM banks (psC's energy tiles are free here) so
                # the five projection groups run back-to-back with no WAR
                # stalls through a single bank
                peA = psC.tile([128, 1024], FP, tag="pe", name="pe")
                peB = psC.tile([128, 1024], FP, tag="pe", name="pe")
                PRO = [("k", 0, None), ("q", 0, peA[:, 0:512]),
                       ("v", 0, peA[:, 512:1024]), ("k", 1, peB[:, 0:512]),
                       ("v", 1, peB[:, 512:1024])]
                for which, j, ps in PRO:
                    for e in range(NE):
                        if ps is None:
                            proj_mm(which, j, e)
                            continue
                        nc.tensor.matmul(ps, W_OF[which][:, e * CH:(e + 1) * CH],
                                         xt(j, e), start=(e == 0),
                                         stop=(e == NE - 1))
                        if e == NE - 1:
                            bi = B_OF[which]
                            nc.vector.tensor_scalar_add(
                                DST[which][:, j * 512:j * 512 + 512], ps,
                                bias_sb[:, bi:bi + 1])
                    if which == "v":
                        for si in range(4):
                            vtr(j, si)

                # bulk DMAs: transfers serialize on the DMA engines in issue
                # order; everything here lands well before its consumer
                nc.sync.dma_start(x_sb[:, xs(2, 0, 8)], xP[:, xs(2, 0, 8)])
                nc.sync.dma_start(x_sb[:, xs(3, 0, 8)], xP[:, xs(3, 0, 8)])
                nc.sync.dma_start(x_sb[:, xs(4, 0, 8)], xP[:, xs(4, 0, 8)])
                nc.sync.dma_start(x_sb[:, xs(5, 0, 8)], xP[:, xs(5, 0, 8)])
                nc.sync.dma_start(x_sb[:, xs(6, 0, 8)], xP[:, xs(6, 0, 8)])
                nc.sync.dma_start(x_sb[:, xs(7, 0, 8)], xP[:, xs(7, 0, 8)])
                nc.sync.dma_start(wo_sb[:], woP)
                nc.sync.dma_start(bo_sb[:], boB)

                # per-block proj work: block b hosts KV(j_{b+3}) for b=1..4,
                # block 0 hosts KV(j2)+KV(j3), block b hosts Q(j_{b+1}) b=0..6
                PROJ_OF = {
                    0: proj_units(("k", 2), ("v", 2), ("q", 1),
                                  ("k", 3), ("v", 3)),
                    1: proj_units(("k", 4), ("v", 4), ("q", 2)),
                    2: proj_units(("k", 5), ("v", 5), ("q", 3)),
                    3: proj_units(("k", 6), ("v", 6), ("q", 4)),
                    4: proj_units(("k", 7), ("v", 7), ("q", 5)),
                    5: proj_units(("q", 6)),
                    6: proj_units(("q", 7)),
                    7: [],
                }

                pending = None  # previous block's deferred tail
                for b in range(NB * 4):
                    n = b // 4
                    q0 = b * 512
                    first = pending is None
                    po0 = psO.tile([65, 512], FP, tag="o0", name="po0")
                    po1 = psO.tile([65, 512], FP, tag="o1", name="po1")
                    pts = {}
                    proj_q = list(PROJ_OF[b])

                    def av(ik, n=n, po0=po0, po1=po1, pts=pts):
                        # attn@V plus denominator (ones column) in one
                        # M=65 matmul per head
                        vb = (n * 16 + ik) * 130
                        ptp = pts.pop(ik)
                        nc.tensor.matmul(
                            po0[0:65, :], V_sb[:, vb:vb + 65],
                            ptp[:, 0:512],
                            start=(ik == 0), stop=(ik == 15))
                        nc.tensor.matmul(
                            po1[0:65, :], V_sb[:, vb + 65:vb + 130],
                            ptp[:, 512:1024],
                            start=(ik == 0), stop=(ik == 15))

                    st = {"av": 0}

                    def pop(ik):
                        # one PE filler: priority proj > av catch-up
                        if proj_q:
                            u = proj_q.pop(0)
                            if u is not None:
                                u()
                                return True
                            # None spacer: give the evict WAR a slot; fall
                            # through to an av for this slot instead
                        # avs lag the exp stream by >=1 ik (pt must be done
                        # so the in-order PE queue never blocks on ACT);
                        # av15 is always deferred to the next block's ik0
                        if st["av"] <= min(ik - 1, 14) and st["av"] in pts \
                                and (first or ik >= 6):
                            av(st["av"])
                            st["av"] += 1
                            return True
                        return False

                    for ik in range(16):
                        k0 = n * S + ik * 128
                        pe = psC.tile([128, 1024], FP, tag="pe", name="pe")
                        nc.tensor.matmul(
                            pe[:, 0:512],
                            KT_sb[0:HD, k0:k0 + 128],
                            QT_sb[0:HD, q0:q0 + 512],
                            start=True, stop=True)
                        # one filler between the energy pair (keeps the
                        # row-tiled pair non-adjacent in the PE queue)
                        if not first and ik == 0:
                            pending["av15"]()
                        else:
                            pop(ik)
                        nc.tensor.matmul(
                            pe[:, 512:1024],
                            KT_sb[HD:128, k0:k0 + 128],
                            QT_sb[HD:128, q0:q0 + 512],
                            start=True, stop=True)
                        pt = ptpool.tile([128, 1024], BF, tag="pt", name="pt")
                        nc.scalar.activation(pt[:], pe[:], AF.Exp, scale=0.125)
                        pts[ik] = pt
                        # post-exp work (off the exp critical path)
                        if not first:
                            if ik == 0:
                                nc.vector.reciprocal(
                                    pending["rd0"][:],
                                    pending["po0"][64:65, :])
                                nc.vector.reciprocal(
                                    pending["rd1"][:],
                                    pending["po1"][64:65, :])
                            elif ik == 1:
                                # replicate reciprocals across partitions on
                                # the (idle) gpsimd engine; both broadcasts
                                # write base-partition-0 tiles (HW ucode does
                                # not honor partition-offset outputs)
                                rep0 = rpool.tile([64, 512], BF,
                                                  tag="rep0", name="rep0")
                                rep1 = rpool.tile([64, 512], BF,
                                                  tag="rep1", name="rep1")
                                nc.gpsimd.partition_broadcast(
                                    rep0[:], pending["rd0"][:])
                                nc.gpsimd.partition_broadcast(
                                    rep1[:], pending["rd1"][:])
                                pending["rep"] = (rep0, rep1)
                            elif ik == 3:
                                rep0, rep1 = pending["rep"]
                                attb = rpool.tile([128, 512], BF,
                                                  tag="attb", name="attb")
                                nc.vector.tensor_mul(
                                    attb[0:64, :],
                                    pending["po0"][0:64, :], rep0[:])
                                nc.vector.tensor_mul(
                                    attb[64:128, :],
                                    pending["po1"][0:64, :], rep1[:])
                                nc.sync.dma_start(
                                    bounce_in[pending["b"]], attb[:])
                        # up to three more fillers after the exp issue
                        for _ in range(3):
                            if not pop(ik):
                                break

                    # flush: everything except av15 must be emitted in-block
                    for u in proj_q:
                        if u is not None:
                            u()
                    while st["av"] <= 14:
                        av(st["av"])
                        st["av"] += 1

                    pending = {
                        "av15": (lambda av=av: av(15)),
                        "po0": po0, "po1": po1, "b": b,
                        "rd0": rpool.tile([1, 512], BF, tag="rd0",
                                          name="rd0"),
                        "rd1": rpool.tile([1, 512], BF, tag="rd1",
                                          name="rd1"),
                    }
                # final block tail
                pending["av15"]()
                nc.vector.reciprocal(pending["rd0"][:], pending["po0"][64:65, :])
                nc.vector.reciprocal(pending["rd1"][:], pending["po1"][64:65, :])
                rep0 = rpool.tile([64, 512], BF, tag="rep0", name="rep0")
                rep1 = rpool.tile([64, 512], BF, tag="rep1", name="rep1")
                nc.gpsimd.partition_broadcast(rep0[:], pending["rd0"][:])
                nc.gpsimd.partition_broadcast(rep1[:], pending["rd1"][:])
                attb = rpool.tile([128, 512], BF, tag="attb", name="attb")
                nc.vector.tensor_mul(attb[0:64, :], pending["po0"][0:64, :],
                                     rep0[:])
                nc.vector.tensor_mul(attb[64:128, :], pending["po1"][0:64, :],
                                     rep1[:])
                nc.sync.dma_start(bounce_in[pending["b"]], attb[:])

            # ================= AllToAll redistribution =================
            if local_only:
                nc.sync.dma_start(bounce_out[:], bounce_in[:])
            else:
                nc.gpsimd.collective_compute(
                    "AllToAll", mybir.AluOpType.bypass,
                    ins=[bounce_in.opt()], outs=[bounce_out.opt()],
                    replica_groups=[list(range(N_CORES))],
                )

            # ====== Phase D: output projection, transposed (tok on part) ====
            with tc.tile_pool(name="psD", bufs=2, space="PSUM") as psD, \
                 tc.tile_pool(name="dsb", bufs=1) as dpool, \
                 tc.tile_pool(name="ybuf", bufs=2) as ypool:
                att_sb = dpool.tile([128, NE * 512], BF, tag="att")
                att3 = att_sb.rearrange("p (i t) -> p i t", i=NE)
                bo3 = bounce_out.rearrange("i p t -> p i t")
                for tk in range(4):
                    # per-tk unpack: phase D starts after 1/4 of the exchange
                    nc.sync.dma_start(att3[:, :, tk * 128:(tk + 1) * 128],
                                      bo3[:, :, tk * 128:(tk + 1) * 128])
                for tk in range(4):
                    y_sb = ypool.tile([128, EMBED], BF, tag="y_sb")
                    py0 = psD.tile([128, 512], FP, tag="y0")
                    py1 = psD.tile([128, 512], FP, tag="y1")
                    # ci outer / half inner: one att LDWEIGHTS serves both
                    # 512-col output halves
                    for ci in range(NE):
                        a = att_sb[:, ci * 512 + tk * 128:ci * 512 + (tk + 1) * 128]
                        wo_ci = wo_sb[:, ci * EMBED:(ci + 1) * EMBED]
                        nc.tensor.matmul(py0[:], a, wo_ci[:, 0:512],
                                         start=(ci == 0), stop=(ci == NE - 1))
                        nc.tensor.matmul(py1[:], a, wo_ci[:, 512:1024],
                                         start=(ci == 0), stop=(ci == NE - 1))
                    nc.vector.tensor_add(y_sb[:, 0:512], py0[:],
                                         bo_sb[:, 0:512])
                    nc.vector.tensor_add(y_sb[:, 512:1024], py1[:],
                                         bo_sb[:, 512:1024])
                    nc.sync.dma_start(Y[tk * 128:(tk + 1) * 128, :], y_sb[:])
                if dbg:
                    nc.sync.dma_start(DQT, QT_sb[:])
                    nc.sync.dma_start(DKT, KT_sb[:])
                    nc.sync.dma_start(DVS, V_sb[:])
                    nc.sync.dma_start(DBIN, bounce_in[:])
                    nc.sync.dma_start(DATT, att_sb[:])
            nc._dbg = {"QT": QT_sb, "KT": KT_sb, "VT": VT_sb, "V": V_sb,
                       "bin": bounce_in, "att": att_sb, "x": x_sb}
    nc.compile()
    return nc


def _prep_inputs(x, Wq, bq, Aq, Bq, Wk, bk, Ak, Bk, Wv, bv, Av, Bv, Wo, bo, Ao, Bo):
    f32 = np.float32
    f64 = np.float64
    xT = x.reshape(T, EMBED).T.astype(f32)          # [1024, 4096]
    # [128, (j e t)]: row p, col ((j*8+e)*512 + t) = xT[e*128+p, j*512+t]
    xPm = np.ascontiguousarray(
        xT.reshape(NE, 128, NJ, 512).transpose(1, 2, 0, 3).reshape(128, -1)
    ).astype(BF_NP)
    # fold LoRA into the dense weights (exact algebra)
    Wq_eff = (Wq.astype(f64) + 2.0 * Bq.astype(f64) @ Aq.astype(f64)).astype(f32)
    Wk_eff = (Wk.astype(f64) + 2.0 * Bk.astype(f64) @ Ak.astype(f64)).astype(f32)
    Wv_eff = (Wv.astype(f64) + 2.0 * Bv.astype(f64) @ Av.astype(f64)).astype(f32)
    Wo_eff = (Wo.astype(f64) + 2.0 * Bo.astype(f64) @ Ao.astype(f64)).astype(f32)

    def wprep(Weff, sl):
        # W.T slice [1024, 128] -> [128, (e c)]
        wT = Weff[sl, :].T.astype(f32)              # [1024, 128]
        return np.ascontiguousarray(
            wT.reshape(NE, 128, CH).transpose(1, 0, 2).reshape(128, -1)
        ).astype(BF_NP)

    identm = np.eye(128, dtype=f32).astype(BF_NP)
    # [128, (ci o)]: [p, ci*1024+o] = Wo_eff.T[ci*128+p, o]
    woPm = np.ascontiguousarray(
        Wo_eff.T.reshape(NE, 128, EMBED).transpose(1, 0, 2).reshape(128, -1)
    ).astype(BF_NP)
    boBm = np.ascontiguousarray(
        np.broadcast_to(bo.reshape(1, EMBED), (128, EMBED)).astype(BF_NP))
    in_maps = []
    for c in range(N_CORES):
        sl = slice(c * CH, (c + 1) * CH)
        bias3m = np.stack([bq[sl], bk[sl], bv[sl]], axis=1).astype(f32)
        wPm = np.concatenate([wprep(Wk_eff, sl), wprep(Wq_eff, sl),
                              wprep(Wv_eff, sl)], axis=1)
        in_maps.append({
            "xP": xPm,
            "wP": np.ascontiguousarray(wPm),
            "bias3": np.ascontiguousarray(bias3m),
            "ident": identm,
            "woP": woPm,
            "boB": boBm,
        })
    return in_maps


def get_nc():
    if "nc" not in _CACHE:
        _CACHE["nc"] = _build()
    return _CACHE["nc"]


def kernel(**inputs) -> np.ndarray:
    nc = get_nc()
    in_maps = _prep_inputs(**{k: np.asarray(v) for k, v in inputs.items()})
    res = bass_utils.run_bass_kernel_spmd(
        nc, in_maps, core_ids=list(range(N_CORES)))
    y = np.concatenate([np.asarray(res.results[c]["Y"], dtype=np.float32)
                        for c in range(N_CORES)], axis=0)
    return np.ascontiguousarray(y).reshape(NB, S, EMBED)


if __name__ == "__main__":
    nc = get_nc()
    print("build+compile OK")


# revision 47
# speedup vs baseline: 1.4704x; 1.4704x over previous
"""LoRA self-attention Trainium2 kernel, 8-way head/tensor parallel.

Single software-pipelined stream (no separate projection phase): the ACT
exp stream (16 exps of [128,1024] per block, ~1.0us each) is the pacing
resource in steady state; everything else fills PE/DVE/DMA slack around it.

Sharding: core c owns heads 2c, 2c+1 (channels 128c..128c+128) for the
QKV projections and attention; the output projection is token-sharded
(core c computes all 1024 output channels for tokens 512c..512c+512)
after an AllToAll exchange of the attention output.

Design:
- LoRA folded into the dense weights on host (W_eff = W + 2*B@A, exact).
- Prologue projects K,Q,V(j0)+K,V(j1) in parallel PSUM banks as soon as
  the first x chunks land; the attention block loop starts right after.
- Remaining projections (KV j2..j7, Q j1..j7) are JIT-interleaved into the
  block loop's PE slots through a single rotating PSUM bank (K->V->Q).
- Softmax denominator free via the ones column in the augmented-V layout
  (M=65 AV matmuls: attn@V in psum partitions 0..63, denominator in 64).
- Block normalization deferred into the next block: reciprocals on DVE,
  replicated across partitions via gpsimd partition_broadcast (frees the
  PSUM bank the old replicate-matmul needed), multiply on DVE, ship.
- bf16 everywhere (fp8 was measured to blow the 2e-2 error budget).
- Output projection computed transposed (tokens on partitions), bias added
  from a host-broadcast [128,1024] tile on DVE during PSUM eviction.
"""
import sys

for p in ("/opt/trn_rl_repo",):
    if p not in sys.path:
        sys.path.append(p)

import numpy as np

import concourse.bass as bass  # noqa: F401
import concourse.tile as tile
from concourse import bacc, mybir
from concourse import bass_utils

N_CORES = 8
EMBED = 1024
HEADS = 16
HD = 64            # head dim
NB = 2             # batch
S = 2048           # seq len
T = NB * S         # 4096 tokens
CH = EMBED // N_CORES  # 128 channels (2 heads) per core
FP = mybir.dt.float32
BF = mybir.dt.bfloat16
AF = mybir.ActivationFunctionType
BF_NP = mybir.dt.np(mybir.dt.bfloat16)

_CACHE: dict = {}

NE = EMBED // 128  # 8 contraction tiles
NJ = T // 512      # 8 token tiles


def _build(local_only=False, dbg=False):
    nc = bacc.Bacc("TRN2", target_bir_lowering=False, debug=False,
                   enable_asserts=False, num_devices=N_CORES)
    if dbg:
        DQT = nc.dram_tensor("DQT", [128, T], BF, kind="ExternalOutput").ap()
        DKT = nc.dram_tensor("DKT", [128, T], BF, kind="ExternalOutput").ap()
        DVS = nc.dram_tensor("DVS", [128, 32 * 130], BF, kind="ExternalOutput").ap()
        DBIN = nc.dram_tensor("DBIN", [N_CORES, 128, 512], BF, kind="ExternalOutput").ap()
        DATT = nc.dram_tensor("DATT", [128, NE * 512], BF, kind="ExternalOutput").ap()
    # ---- DRAM I/O (per-core) ----
    # x pre-arranged on host: [128, (j e t)] = [128, 8*8*512]
    xP = nc.dram_tensor("xP", [128, NJ * NE * 512], BF, kind="ExternalInput").ap()
    # packed weights [wk | wq | wv] each [128, (e c)] = [128, 8*128]
    wP = nc.dram_tensor("wP", [128, 3 * NE * CH], BF, kind="ExternalInput").ap()
    bias3 = nc.dram_tensor("bias3", [CH, 3], FP, kind="ExternalInput").ap()
    ident = nc.dram_tensor("ident", [128, 128], BF, kind="ExternalInput").ap()
    # Wo pre-packed on host: [128, (ci o)] with [p, ci*1024+o] = Wo_eff.T[ci*128+p, o]
    woP = nc.dram_tensor("woP", [128, NE * EMBED], BF, kind="ExternalInput").ap()
    # output bias broadcast to all 128 partitions on host
    boB = nc.dram_tensor("boB", [128, EMBED], BF, kind="ExternalInput").ap()
    Y = nc.dram_tensor("Y", [512, EMBED], BF, kind="ExternalOutput").ap()

    with tile.TileContext(nc) as tc, \
         nc.allow_low_precision(reason="bf16 rounding is intentional"):
        with tc.tile_pool(name="const", bufs=1) as cpool, \
             tc.tile_pool(name="big", bufs=1) as bigpool, \
             tc.tile_pool(name="dram", bufs=1, space="DRAM") as dram:

            # ---- resident tiles ----
            w_all = cpool.tile([128, 3 * NE * CH], BF, tag="wall")
            wk_sb = w_all[:, 0:NE * CH]
            wq_sb = w_all[:, NE * CH:2 * NE * CH]
            wv_sb = w_all[:, 2 * NE * CH:3 * NE * CH]
            x_sb = bigpool.tile([128, NJ * NE * 512], BF, tag="x")
            bias_sb = cpool.tile([CH, 3], FP, tag="bias3")
            id_sb = cpool.tile([128, 128], BF, tag="ident")
            bo_sb = cpool.tile([128, EMBED], BF, tag="boB")
            wo_sb = cpool.tile([128, NE * EMBED], BF, tag="wo")

            QT_sb = bigpool.tile([CH, T], BF, tag="QT")
            KT_sb = bigpool.tile([CH, T], BF, tag="KT")
            VT_sb = bigpool.tile([CH, T], BF, tag="VT")
            # V in [token, ch] layout, 32 strips of [128, 130]:
            # cols [s*130+h*65 : +64] = V head h, col [s*130+h*65+64] = ones
            V_sb = bigpool.tile([128, 32 * 130], BF, tag="Vaug")

            def xs(j, e0, e1):
                return slice((j * NE + e0) * 512, (j * NE + e1) * 512)

            def xt(j, e):
                return x_sb[:, (j * NE + e) * 512:(j * NE + e + 1) * 512]

            # ---- head DMAs: feed the first matmuls ASAP ----
            nc.sync.dma_start(id_sb[:], ident)
            nc.sync.dma_start(w_all[:, 0:NE * CH], wP[:, 0:NE * CH])          # wk
            nc.sync.dma_start(x_sb[:, xs(0, 0, 4)], xP[:, xs(0, 0, 4)])
            nc.sync.dma_start(bias_sb[:], bias3)
            nc.sync.dma_start(w_all[:, NE * CH:2 * NE * CH],
                              wP[:, NE * CH:2 * NE * CH])                     # wq
            nc.sync.dma_start(x_sb[:, xs(0, 4, 8)], xP[:, xs(0, 4, 8)])
            nc.sync.dma_start(w_all[:, 2 * NE * CH:3 * NE * CH],
                              wP[:, 2 * NE * CH:3 * NE * CH])                 # wv
            nc.sync.dma_start(x_sb[:, xs(1, 0, 8)], xP[:, xs(1, 0, 8)])

            # ones columns of the augmented-V layout (all strips, once)
            v_ones = V_sb.rearrange("p (s c) -> p s c", c=65)[:, :, 64]
            nc.vector.memset(v_ones, 1.0)

            bounce_in = dram.tile([N_CORES, 128, 512], BF)
            bounce_out = dram.tile([N_CORES, 128, 512], BF)

            with tc.tile_pool(name="psC", bufs=2, space="PSUM") as psC, \
                 tc.tile_pool(name="psO", bufs=1, space="PSUM") as psO, \
                 tc.tile_pool(name="psP", bufs=1, space="PSUM") as psP, \
                 tc.tile_pool(name="psT", bufs=1, space="PSUM") as psT, \
                 tc.tile_pool(name="pt", bufs=12) as ptpool, \
                 tc.tile_pool(name="rs", bufs=2) as rpool:

                # shared transpose scratch: 8 slots of [128,128]bf16 in one
                # PSUM bank; strip t uses slot t%8 so adjacent j-tiles don't
                # collide
                tr_ps = psT.tile([128, 1024], BF, tag="tr")

                # ---------- projection machinery (shared PSUM bank) ----------
                W_OF = {"k": wk_sb, "q": wq_sb, "v": wv_sb}
                B_OF = {"q": 0, "k": 1, "v": 2}
                DST = {"k": KT_sb, "q": QT_sb, "v": VT_sb}
                pp = {"t": None}

                def proj_mm(which, j, e):
                    if e == 0:
                        pp["t"] = psP.tile([CH, 512], FP, tag="p", name="pp")
                    w = W_OF[which]
                    nc.tensor.matmul(pp["t"][:], w[:, e * CH:(e + 1) * CH],
                                     xt(j, e), start=(e == 0), stop=(e == NE - 1))
                    if e == NE - 1:
                        t0 = j * 512
                        nc.vector.tensor_scalar_add(
                            DST[which][:, t0:t0 + 512], pp["t"][:],
                            bias_sb[:, B_OF[which]:B_OF[which] + 1])

                def vtr(j, si):
                    # one V strip -> augmented [token, ch] layout: PE-mode
                    # transpose into the shared psT slot, two DVE copies out
                    t = j * 4 + si
                    sl = (t % 8) * 128
                    trp = tr_ps[:, sl:sl + 128]
                    nc.tensor.transpose(trp, VT_sb[:, t * 128:(t + 1) * 128],
                                        id_sb[:])
                    base = t * 130
                    nc.vector.tensor_copy(V_sb[:, base:base + 64], trp[:, 0:64])
                    nc.vector.tensor_copy(V_sb[:, base + 65:base + 129],
                                          trp[:, 64:128])

                def proj_units(*specs, defer_vtr=False):
                    # specs: (which, j) -> per-e emitters; None spacers after
                    # each group let the PSUM-bank WAR (evict on DVE) clear
                    # before the next group's start=True matmul. V groups are
                    # followed by the 4 strip transposes; when the strips
                    # aren't needed until a later block (batch-1 tiles),
                    # defer them to the end of the block's unit list so the
                    # transpose's wait on the eviction never stalls the PE
                    # queue mid-stream.
                    out = []
                    tail = []
                    for which, j in specs:
                        for e in range(NE):
                            out.append(lambda which=which, j=j, e=e:
                                       proj_mm(which, j, e))
                        out.append(None)
                        if which == "v":
                            vs = [lambda j=j, si=si: vtr(j, si)
                                  for si in range(4)]
                            if defer_vtr:
                                tail += vs
                            else:
                                out.append(None)
                                out += vs
                    return out + tail

                # warm up the PE clock (p-state/HAM ramp) during the initial
                # x-DMA wait: junk identity matmuls, discarded by the first
                # projection's start=True overwrite
                for _ in range(10):
                    wp = psP.tile([CH, 512], FP, tag="p", name="pp")
                    nc.tensor.matmul(wp[:, 0:128], id_sb[:], id_sb[:],
                                     start=True, stop=True)

                # ---------- prologue: j0 (K,Q,V) + j1 (K,V) ----------
                # parallel PSUM banks (psC's energy tiles are free here) so
                # the five projection groups run back-to-back with no WAR
                # stalls through a single bank
                peA = psC.tile([128, 1024], FP, tag="pe", name="pe")
                peB = psC.tile([128, 1024], FP, tag="pe", name="pe")
                PRO = [("k", 0, None), ("q", 0, peA[:, 0:512]),
                       ("v", 0, peA[:, 512:1024]), ("k", 1, peB[:, 0:512]),
                       ("v", 1, peB[:, 512:1024])]
                for which, j, ps in PRO:
                    for e in range(NE):
                        if ps is None:
                            proj_mm(which, j, e)
                            continue
                        nc.tensor.matmul(ps, W_OF[which][:, e * CH:(e + 1) * CH],
                                         xt(j, e), start=(e == 0),
                                         stop=(e == NE - 1))
                        if e == NE - 1:
                            bi = B_OF[which]
                            nc.vector.tensor_scalar_add(
                                DST[which][:, j * 512:j * 512 + 512], ps,
                                bias_sb[:, bi:bi + 1])
                    if which == "v":
                        for si in range(4):
                            vtr(j, si)

                # bulk DMAs: transfers serialize on the DMA engines in issue
                # order; everything here lands well before its consumer
                nc.sync.dma_start(x_sb[:, xs(2, 0, 8)], xP[:, xs(2, 0, 8)])
                nc.sync.dma_start(x_sb[:, xs(3, 0, 8)], xP[:, xs(3, 0, 8)])
                nc.sync.dma_start(x_sb[:, xs(4, 0, 8)], xP[:, xs(4, 0, 8)])
                nc.sync.dma_start(x_sb[:, xs(5, 0, 8)], xP[:, xs(5, 0, 8)])
                nc.sync.dma_start(x_sb[:, xs(6, 0, 8)], xP[:, xs(6, 0, 8)])
                nc.sync.dma_start(x_sb[:, xs(7, 0, 8)], xP[:, xs(7, 0, 8)])
                nc.sync.dma_start(wo_sb[:], woP)
                nc.sync.dma_start(bo_sb[:], boB)

                # per-block proj work: block b hosts KV(j_{b+3}) for b=1..4,
                # block 0 hosts KV(j2)+KV(j3), block b hosts Q(j_{b+1}) b=0..6
                PROJ_OF = {
                    0: proj_units(("k", 2), ("v", 2), ("q", 1),
                                  ("k", 3), ("v", 3)),
                    1: proj_units(("k", 4), ("v", 4), ("q", 2),
                                  defer_vtr=True),
                    2: proj_units(("k", 5), ("v", 5), ("q", 3),
                                  defer_vtr=True),
                    3: proj_units(("k", 6), ("v", 6), ("q", 4),
                                  defer_vtr=True),
                    4: proj_units(("k", 7), ("v", 7), ("q", 5),
                                  defer_vtr=True),
                    5: proj_units(("q", 6)),
                    6: proj_units(("q", 7)),
                    7: [],
                }

                pending = None  # previous block's deferred tail
                for b in range(NB * 4):
                    n = b // 4
                    q0 = b * 512
                    first = pending is None
                    po0 = psO.tile([65, 512], FP, tag="o0", name="po0")
                    po1 = psO.tile([65, 512], FP, tag="o1", name="po1")
                    pts = {}
                    proj_q = list(PROJ_OF[b])

                    def av(ik, n=n, po0=po0, po1=po1, pts=pts):
                        # attn@V plus denominator (ones column) in one
                        # M=65 matmul per head
                        vb = (n * 16 + ik) * 130
                        ptp = pts.pop(ik)
                        nc.tensor.matmul(
                            po0[0:65, :], V_sb[:, vb:vb + 65],
                            ptp[:, 0:512],
                            start=(ik == 0), stop=(ik == 15))
                        nc.tensor.matmul(
                            po1[0:65, :], V_sb[:, vb + 65:vb + 130],
                            ptp[:, 512:1024],
                            start=(ik == 0), stop=(ik == 15))

                    st = {"av": 0}

                    def pop(ik):
                        # one PE filler: priority proj > av catch-up.
                        # Returns the filler's cost in matmul-equivalents so
                        # the caller can keep per-ik PE filler time below the
                        # ACT exp period (an av is TWO matmuls).
                        if proj_q:
                            u = proj_q.pop(0)
                            if u is not None:
                                u()
                                return 1
                            # None spacer: give the evict WAR a slot; fall
                            # through to an av for this slot instead
                        # avs lag the exp stream (first block is PE-bound so
                        # lag 1 is safe; steady blocks are ACT-bound, lag 2
                        # keeps the in-order PE queue off the ACT tail);
                        # av15 is always deferred to the next block's ik0
                        lag = 1 if first else 2
                        if st["av"] <= min(ik - lag, 14) and st["av"] in pts \
                                and (first or ik >= 5):
                            av(st["av"])
                            st["av"] += 1
                            return 2
                        return 0

                    for ik in range(16):
                        k0 = n * S + ik * 128
                        pe = psC.tile([128, 1024], FP, tag="pe", name="pe")
                        nc.tensor.matmul(
                            pe[:, 0:512],
                            KT_sb[0:HD, k0:k0 + 128],
                            QT_sb[0:HD, q0:q0 + 512],
                            start=True, stop=True)
                        # energy pair back-to-back (on HW the two row-tiled
                        # halves run concurrently); ALL fillers go after the
                        # exp issue so the exp stream is never delayed
                        nc.tensor.matmul(
                            pe[:, 512:1024],
                            KT_sb[HD:128, k0:k0 + 128],
                            QT_sb[HD:128, q0:q0 + 512],
                            start=True, stop=True)
                        pt = ptpool.tile([128, 1024], BF, tag="pt", name="pt")
                        nc.scalar.activation(pt[:], pe[:], AF.Exp, scale=0.125)
                        pts[ik] = pt
                        # post-exp work (off the exp critical path)
                        if not first:
                            if ik == 0:
                                pending["av15"]()
                                nc.vector.reciprocal(
                                    pending["rd0"][:],
                                    pending["po0"][64:65, :])
                                nc.vector.reciprocal(
                                    pending["rd1"][:],
                                    pending["po1"][64:65, :])
                            elif ik == 1:
                                # replicate reciprocals across partitions on
                                # the (idle) gpsimd engine; both broadcasts
                                # write base-partition-0 tiles (HW ucode does
                                # not honor partition-offset outputs)
                                rep0 = rpool.tile([64, 512], BF,
                                                  tag="rep0", name="rep0")
                                rep1 = rpool.tile([64, 512], BF,
                                                  tag="rep1", name="rep1")
                                nc.gpsimd.partition_broadcast(
                                    rep0[:], pending["rd0"][:])
                                nc.gpsimd.partition_broadcast(
                                    rep1[:], pending["rd1"][:])
                                pending["rep"] = (rep0, rep1)
                            elif ik == 3:
                                rep0, rep1 = pending["rep"]
                                attb = rpool.tile([128, 512], BF,
                                                  tag="attb", name="attb")
                                nc.vector.tensor_mul(
                                    attb[0:64, :],
                                    pending["po0"][0:64, :], rep0[:])
                                nc.vector.tensor_mul(
                                    attb[64:128, :],
                                    pending["po1"][0:64, :], rep1[:])
                                nc.sync.dma_start(
                                    bounce_in[pending["b"]], attb[:])
                        # fillers after the exp issue, capped at ~3 matmuls
                        # so the next energy pair is never starved; the first
                        # block is PE-bound anyway, so drain backlog freely
                        budget = 6 if first else 3
                        while budget > 0:
                            c = pop(ik)
                            if c == 0:
                                break
                            budget -= c

                    # flush: everything except av15 must be emitted in-block
                    for u in proj_q:
                        if u is not None:
                            u()
                    while st["av"] <= 14:
                        av(st["av"])
                        st["av"] += 1

                    pending = {
                        "av15": (lambda av=av: av(15)),
                        "po0": po0, "po1": po1, "b": b,
                        "rd0": rpool.tile([1, 512], BF, tag="rd0",
                                          name="rd0"),
                        "rd1": rpool.tile([1, 512], BF, tag="rd1",
                                          name="rd1"),
                    }
                # final block tail
                pending["av15"]()
                nc.vector.reciprocal(pending["rd0"][:], pending["po0"][64:65, :])
                nc.vector.reciprocal(pending["rd1"][:], pending["po1"][64:65, :])
                rep0 = rpool.tile([64, 512], BF, tag="rep0", name="rep0")
                rep1 = rpool.tile([64, 512], BF, tag="rep1", name="rep1")
                nc.gpsimd.partition_broadcast(rep0[:], pending["rd0"][:])
                nc.gpsimd.partition_broadcast(rep1[:], pending["rd1"][:])
                attb = rpool.tile([128, 512], BF, tag="attb", name="attb")
                nc.vector.tensor_mul(attb[0:64, :], pending["po0"][0:64, :],
                                     rep0[:])
                nc.vector.tensor_mul(attb[64:128, :], pending["po1"][0:64, :],
                                     rep1[:])
                nc.sync.dma_start(bounce_in[pending["b"]], attb[:])

                # ============ AllToAll redistribution ============
                # (same pool scope: no pool-close drain stalls the PE queue
                # between the attention stream and the output projection)
                if local_only:
                    nc.sync.dma_start(bounce_out[:], bounce_in[:])
                else:
                    nc.gpsimd.collective_compute(
                        "AllToAll", mybir.AluOpType.bypass,
                        ins=[bounce_in.opt()], outs=[bounce_out.opt()],
                        replica_groups=[list(range(N_CORES))],
                    )

                # == Phase D: output projection, transposed (tok on part) ==
                att_sb = bigpool.tile([128, NE * 512], BF, tag="att")
                att3 = att_sb.rearrange("p (i t) -> p i t", i=NE)
                bo3 = bounce_out.rearrange("i p t -> p i t")
                for tk in range(4):
                    # per-tk unpack: phase D starts after 1/4 of the exchange
                    nc.sync.dma_start(att3[:, :, tk * 128:(tk + 1) * 128],
                                      bo3[:, :, tk * 128:(tk + 1) * 128])
                # keep the PE clock warm across the collective+unpack idle:
                # dependency-free junk matmuls bridge the gap so the real
                # output-projection matmuls start at full clock
                for _ in range(32):
                    pw = psC.tile([128, EMBED], FP, tag="pe", name="pe")
                    nc.tensor.matmul(pw[:, 0:512], id_sb[:], w_all[:, 0:512],
                                     start=True, stop=True)
                    nc.tensor.matmul(pw[:, 512:1024], id_sb[:],
                                     w_all[:, 512:1024], start=True, stop=True)
                for tk in range(4):
                    y_sb = rpool.tile([128, EMBED], BF, tag="y_sb")
                    py = psC.tile([128, EMBED], FP, tag="pe", name="pe")
                    # ci outer / half inner (the ISA caps a matmul's moving
                    # free size at 512): one att LDWEIGHTS serves both halves
                    for ci in range(NE):
                        a = att_sb[:, ci * 512 + tk * 128:ci * 512 + (tk + 1) * 128]
                        wo_ci = wo_sb[:, ci * EMBED:(ci + 1) * EMBED]
                        nc.tensor.matmul(py[:, 0:512], a, wo_ci[:, 0:512],
                                         start=(ci == 0), stop=(ci == NE - 1))
                        nc.tensor.matmul(py[:, 512:1024], a, wo_ci[:, 512:1024],
                                         start=(ci == 0), stop=(ci == NE - 1))
                    nc.vector.tensor_add(y_sb[:], py[:], bo_sb[:])
                    nc.sync.dma_start(Y[tk * 128:(tk + 1) * 128, :], y_sb[:])
                if dbg:
                    nc.sync.dma_start(DQT, QT_sb[:])
                    nc.sync.dma_start(DKT, KT_sb[:])
                    nc.sync.dma_start(DVS, V_sb[:])
                    nc.sync.dma_start(DBIN, bounce_in[:])
                    nc.sync.dma_start(DATT, att_sb[:])
            nc._dbg = {"QT": QT_sb, "KT": KT_sb, "VT": VT_sb, "V": V_sb,
                       "bin": bounce_in, "att": att_sb, "x": x_sb}
    nc.compile()
    return nc


def _prep_inputs(x, Wq, bq, Aq, Bq, Wk, bk, Ak, Bk, Wv, bv, Av, Bv, Wo, bo, Ao, Bo):
    f32 = np.float32
    f64 = np.float64
    xT = x.reshape(T, EMBED).T.astype(f32)          # [1024, 4096]
    # [128, (j e t)]: row p, col ((j*8+e)*512 + t) = xT[e*128+p, j*512+t]
    xPm = np.ascontiguousarray(
        xT.reshape(NE, 128, NJ, 512).transpose(1, 2, 0, 3).reshape(128, -1)
    ).astype(BF_NP)
    # fold LoRA into the dense weights (exact algebra)
    Wq_eff = (Wq.astype(f64) + 2.0 * Bq.astype(f64) @ Aq.astype(f64)).astype(f32)
    Wk_eff = (Wk.astype(f64) + 2.0 * Bk.astype(f64) @ Ak.astype(f64)).astype(f32)
    Wv_eff = (Wv.astype(f64) + 2.0 * Bv.astype(f64) @ Av.astype(f64)).astype(f32)
    Wo_eff = (Wo.astype(f64) + 2.0 * Bo.astype(f64) @ Ao.astype(f64)).astype(f32)

    def wprep(Weff, sl):
        # W.T slice [1024, 128] -> [128, (e c)]
        wT = Weff[sl, :].T.astype(f32)              # [1024, 128]
        return np.ascontiguousarray(
            wT.reshape(NE, 128, CH).transpose(1, 0, 2).reshape(128, -1)
        ).astype(BF_NP)

    identm = np.eye(128, dtype=f32).astype(BF_NP)
    # [128, (ci o)]: [p, ci*1024+o] = Wo_eff.T[ci*128+p, o]
    woPm = np.ascontiguousarray(
        Wo_eff.T.reshape(NE, 128, EMBED).transpose(1, 0, 2).reshape(128, -1)
    ).astype(BF_NP)
    boBm = np.ascontiguousarray(
        np.broadcast_to(bo.reshape(1, EMBED), (128, EMBED)).astype(BF_NP))
    in_maps = []
    for c in range(N_CORES):
        sl = slice(c * CH, (c + 1) * CH)
        bias3m = np.stack([bq[sl], bk[sl], bv[sl]], axis=1).astype(f32)
        wPm = np.concatenate([wprep(Wk_eff, sl), wprep(Wq_eff, sl),
                              wprep(Wv_eff, sl)], axis=1)
        in_maps.append({
            "xP": xPm,
            "wP": np.ascontiguousarray(wPm),
            "bias3": np.ascontiguousarray(bias3m),
            "ident": identm,
            "woP": woPm,
            "boB": boBm,
        })
    return in_maps


def get_nc():
    if "nc" not in _CACHE:
        _CACHE["nc"] = _build()
    return _CACHE["nc"]


def kernel(**inputs) -> np.ndarray:
    nc = get_nc()
    in_maps = _prep_inputs(**{k: np.asarray(v) for k, v in inputs.items()})
    res = bass_utils.run_bass_kernel_spmd(
        nc, in_maps, core_ids=list(range(N_CORES)))
    y = np.concatenate([np.asarray(res.results[c]["Y"], dtype=np.float32)
                        for c in range(N_CORES)], axis=0)
    return np.ascontiguousarray(y).reshape(NB, S, EMBED)


if __name__ == "__main__":
    nc = get_nc()
    print("build+compile OK")
